# revision 39
# baseline (speedup 1.0000x reference)
"""Multi-head attention (B=2, S=2048, D=1024, H=16) on 8 TRN2 NeuronCores.

Sharding: core = (batch b, 4 consecutive heads). Each core computes its 4
heads' Q/K/V projections, full [S,S] softmax attention (written to the attn
output), attn@V, and a partial O-projection. Host sums the 4 per-batch
O-partials and adds bo.

Device layout notes:
- All matmul contractions sit on the SBUF partition axis, so the host passes
  transposed views: qt/kt/vt = X[b].T [D,S]; wq/wk/wv = W[rows_h,:].T [D,256];
  wo = Wo[:,cols_h].T [256,D].
- Scores are computed twice, in [q,k] layout (for the attn output; exp row
  sums come free via activation accum_out) and in [k,q] layout (for attn@V;
  row sums come free via a ones-column appended to v).
"""

import sys

sys.path.insert(0, "/opt/trn_rl_repo")

from contextlib import ExitStack

import numpy as np

import concourse.bacc as bacc
import concourse.bass as bass
import concourse.mybir as mybir
import concourse.tile as tile
from concourse.bass_utils import run_bass_kernel_spmd

F32 = mybir.dt.float32
F32R = mybir.dt.float32r
BF16 = mybir.dt.bfloat16
AF = mybir.ActivationFunctionType




B, S, D, H, DK = 2, 2048, 1024, 16, 64
N_CORES = 8
HPC = H * B // N_CORES  # 4 heads per core
PF = HPC * DK  # 256 projected features per core
SCALE = 1.0 / np.sqrt(DK)  # 0.125


def build_program():
    nc = bacc.Bacc("TRN2", target_bir_lowering=False, debug=False, num_devices=N_CORES)

    qt = nc.dram_tensor("qt", [D, S], F32, kind="ExternalInput").ap()
    kt = nc.dram_tensor("kt", [D, S], F32, kind="ExternalInput").ap()
    vt = nc.dram_tensor("vt", [D, S], F32, kind="ExternalInput").ap()
    wq = nc.dram_tensor("wq", [D, PF], F32, kind="ExternalInput").ap()
    wk = nc.dram_tensor("wk", [D, PF], F32, kind="ExternalInput").ap()
    wv = nc.dram_tensor("wv", [D, PF], F32, kind="ExternalInput").ap()
    wo = nc.dram_tensor("wo", [PF, D], F32, kind="ExternalInput").ap()
    bq = nc.dram_tensor("bq", [PF, 1], F32, kind="ExternalInput").ap()
    bk = nc.dram_tensor("bk", [PF, 1], F32, kind="ExternalInput").ap()
    bv = nc.dram_tensor("bv", [128, PF], F32, kind="ExternalInput").ap()
    attn = nc.dram_tensor("attn", [HPC, S, S], F32, kind="ExternalOutput").ap()
    outp = nc.dram_tensor("outp", [S, D], F32, kind="ExternalOutput").ap()

    with tile.TileContext(nc) as tc, ExitStack() as ctx:
        _attention(ctx, tc, qt, kt, vt, wq, wk, wv, wo, bq, bk, bv, attn, outp)

    nc.compile()
    return nc


def _attention(ctx, tc, qt, kt, vt, wq, wk, wv, wo, bq, bk, bv, attn, outp):
    nc = tc.nc
    NDC = D // 128  # 8 contraction chunks for projections

    const = ctx.enter_context(tc.tile_pool(name="const", bufs=1))
    # PSUM budget (8 banks): "scores" 3 x [128,1024] = 6 banks,
    # "avsm" 2 x [128,512] = 2 banks (v-proj, AV accumulators, O-proj).
    ps = ctx.enter_context(tc.tile_pool(name="ps", bufs=3, space="PSUM"))
    ps_sm = ctx.enter_context(tc.tile_pool(name="ps_sm", bufs=2, space="PSUM"))

    # ---------------- persistent tiles ----------------
    wo_r = const.tile([128, 2, D], F32R, name="wo_r")
    bq_sb = const.tile([128, 2, 1], F32, name="bq_sb")
    bk_sb = const.tile([128, 2, 1], F32, name="bk_sb")
    bv_sb = const.tile([128, PF], F32, name="bv_sb")
    nc.sync.dma_start(bq_sb[:], bq.rearrange("(c p) o -> p c o", p=128))
    nc.sync.dma_start(bk_sb[:], bk.rearrange("(c p) o -> p c o", p=128))
    nc.sync.dma_start(bv_sb[:], bv[:])

    # qT/kT: per head-pair p, [128 (2 heads x 64 feat), S] f32r.
    qT = [const.tile([128, S], F32R, name=f"qT{p}") for p in range(2)]
    kT = [const.tile([128, S], F32R, name=f"kT{p}") for p in range(2)]
    # v (+ ones col): per s-chunk of 128, [128 (seq), 4 heads, DK+1] bf16.
    vaug = [const.tile([128, HPC, DK + 1], BF16, name=f"vaug{sc}") for sc in range(16)]

    # ---------------- projections ----------------
    with tc.tile_pool(name="wraw", bufs=1) as wraw, \
         tc.tile_pool(name="wrnd", bufs=1) as wrnd, \
         tc.tile_pool(name="inp", bufs=4) as inp, \
         tc.tile_pool(name="inpr", bufs=12) as inpr:

        def load_round(name, dram_ap, shape, raw_pool=None, rnd_pool=None,
                       tag=None):
            """DMA f32 into a raw tile, round to f32r on the (otherwise idle
            during projections) ScalarEngine."""
            raw_pool = raw_pool or inp
            rnd_pool = rnd_pool or inpr
            raw = raw_pool.tile(shape, F32, name=f"{name}w", tag=tag or f"{name}w")
            nc.sync.dma_start(raw[:], dram_ap)
            rnd = rnd_pool.tile(shape, F32R, name=name, tag=tag or name)
            nc.scalar.copy(rnd[:], raw[:])
            return rnd

        wq_r = load_round("wq_r", wq.rearrange("(c p) d -> p c d", p=128),
                          [128, NDC, PF], wraw, wrnd)
        wk_r = load_round("wk_r", wk.rearrange("(c p) d -> p c d", p=128),
                          [128, NDC, PF], wraw, wrnd)
        wv_r = load_round("wv_r", wv.rearrange("(c p) d -> p c d", p=128),
                          [128, NDC, PF], wraw, wrnd)
        wo_raw = wraw.tile([128, 2, D], F32, name="wo_raw")
        nc.sync.dma_start(wo_raw[:], wo.rearrange("(c p) m -> p c m", p=128))
        nc.scalar.copy(wo_r[:], wo_raw[:])

        # q/k projections, both head pairs at once per input chunk.
        for tsrc, wsb, bsb, dst in ((qt, wq_r, bq_sb, qT), (kt, wk_r, bk_sb, kT)):
            for sc in range(2):  # s chunks of 1024
                xt = [load_round(f"xr{dc}",
                                 tsrc[dc * 128:(dc + 1) * 128,
                                      sc * 1024:(sc + 1) * 1024],
                                 [128, 1024], tag="xr")
                      for dc in range(NDC)]
                for p in range(2):
                    pp = ps.tile([128, 1024], F32, name="pp", tag="scores")
                    for half in range(2):
                        for dc in range(NDC):
                            nc.tensor.matmul(
                                pp[:, half * 512:(half + 1) * 512],
                                wsb[:, dc, p * 128:(p + 1) * 128],
                                xt[dc][:, half * 512:half * 512 + 512],
                                start=(dc == 0), stop=(dc == NDC - 1),
                            )
                    nc.vector.tensor_scalar_add(
                        dst[p][:, sc * 1024:(sc + 1) * 1024], pp[:], bsb[:, p, :])

        # v projection: [s,256] = vt.T @ wv, all 4 heads at once.
        for sc2 in range(2):
            xt = [load_round(f"vr{dc}",
                             vt[dc * 128:(dc + 1) * 128,
                                sc2 * 1024:(sc2 + 1) * 1024],
                             [128, 1024], tag="xr")
                  for dc in range(NDC)]
            for ss in range(8):  # s sub-chunks of 128
                sc = sc2 * 8 + ss
                vp = ps_sm.tile([128, PF], F32, name="vp", tag="avsm")
                for dc in range(NDC):
                    nc.tensor.matmul(
                        vp[:], xt[dc][:, ss * 128:(ss + 1) * 128], wv_r[:, dc, :],
                        start=(dc == 0), stop=(dc == NDC - 1),
                    )
                nc.vector.memset(vaug[sc][:, :, DK:DK + 1], 1.0)
                nc.vector.tensor_tensor(
                    vaug[sc][:, :, 0:DK],
                    vp.rearrange("p (h d) -> p h d", d=DK),
                    bv_sb.rearrange("p (h d) -> p h d", d=DK),
                    op=mybir.AluOpType.add,
                )

    sb = ctx.enter_context(tc.tile_pool(name="sb", bufs=2))
    ebp = ctx.enter_context(tc.tile_pool(name="ebp", bufs=6))
    eap = ctx.enter_context(tc.tile_pool(name="eap", bufs=3))
    vec = ctx.enter_context(tc.tile_pool(name="vec", bufs=4))

    # ---------------- main loop ----------------
    def b_scores(p, c, kc):
        """Packed scores.T for both heads of pair p at k-chunk kc + exp."""
        q0 = c * 512
        sT = ps.tile([128, 1024], F32, name="sT", tag="scores")
        for hi in range(2):
            nc.tensor.matmul(
                sT[:, hi * 512:(hi + 1) * 512],
                kT[p][hi * 64:(hi + 1) * 64, kc * 128:(kc + 1) * 128],
                qT[p][hi * 64:(hi + 1) * 64, q0:q0 + 512],
                start=True, stop=True,
                tile_position=(hi * 64, 0),
            )
        ebT = ebp.tile([128, 2, 512], BF16, name="ebT", tag="ebT")
        nc.scalar.activation(
            ebT[:], sT.rearrange("p (h q) -> p h q", h=2), AF.Exp, scale=SCALE)
        return ebT

    def b_av(p, kc, ebT, av):
        """AV accumulation matmuls for k-chunk kc (one chunk behind scores)."""
        for hi in range(2):
            nc.tensor.matmul(
                av[hi][:], vaug[kc][:, p * 2 + hi, :], ebT[:, hi, :],
                start=(kc == 0), stop=(kc == 15),
            )

    def b_finish(p, c, av, st):
        """Normalize attn@V into the stacked st tile for pair p."""
        for hi in range(2):
            rT = vec.tile([1, 512], F32, name="rT", tag="rT")
            nc.vector.reciprocal(rT[:], av[hi][DK:DK + 1, :])
            rTb = vec.tile([DK, 512], F32, name="rTb", tag="rTb", bufs=2)
            nc.gpsimd.partition_broadcast(rTb[:], rT[:])
            nc.vector.tensor_tensor(
                st[hi * 64:(hi + 1) * 64, :], av[hi][0:DK, :],
                rTb[:], op=mybir.AluOpType.mult)

    def a_block(p, c, qs):
        """A-phase for both heads of pair p at q sub-chunk qs (packed
        matmuls): scores [q,k] -> exp (+rowsum) -> normalize -> attn out."""
        r0 = c * 512 + qs * 128
        ea = [eap.tile([128, S], BF16, name=f"ea{hi}", tag=f"ea{hi}")
              for hi in range(2)]
        rs = [vec.tile([128, 2], F32, name=f"rs{hi}", tag=f"rs{hi}")
              for hi in range(2)]
        for k4 in range(2):  # k chunks of 1024
            sA = [ps.tile([128, 1024], F32, name=f"sA{hi}", tag="scores")
                  for hi in range(2)]
            for kh in range(2):
                k0 = k4 * 1024 + kh * 512
                for hi in range(2):
                    nc.tensor.matmul(
                        sA[hi][:, kh * 512:(kh + 1) * 512],
                        qT[p][hi * 64:(hi + 1) * 64, r0:r0 + 128],
                        kT[p][hi * 64:(hi + 1) * 64, k0:k0 + 512],
                        start=True, stop=True,
                        tile_position=(hi * 64, 0),
                    )
            for hi in range(2):
                nc.scalar.activation(
                    ea[hi][:, k4 * 1024:(k4 + 1) * 1024], sA[hi][:], AF.Exp,
                    scale=SCALE, accum_out=rs[hi][:, k4:k4 + 1])
        for hi in range(2):
            h = p * 2 + hi
            rsum = vec.tile([128, 1], F32, name="rsum", tag="rsum")
            nc.vector.tensor_reduce(
                rsum[:], rs[hi][:], axis=mybir.AxisListType.X,
                op=mybir.AluOpType.add)
            rinv = vec.tile([128, 1], F32, name="rinv", tag="rinv")
            nc.vector.reciprocal(rinv[:], rsum[:])
            at = eap.tile([128, S], F32, name=f"at{hi}", tag=f"at{hi}", bufs=2)
            nc.vector.tensor_scalar_mul(at[:], ea[hi][:], rinv[:])
            nc.sync.dma_start(attn[h, r0:r0 + 128, :], at[:])

    def o_block(c, qs, st):
        """O-projection for q sub-chunk qs: out_partial += sum_p st_p.T@wo_p."""
        q0 = c * 512
        ot = sb.tile([128, D], F32, name="ot", tag="ot")
        for mc in range(2):
            op_ = ps.tile([128, 512], F32, name="op_", tag="scores")
            for p in range(2):
                nc.tensor.matmul(
                    op_[:], st[p][:, qs * 128:(qs + 1) * 128],
                    wo_r[:, p, mc * 512:(mc + 1) * 512],
                    start=(p == 0), stop=(p == 1),
                )
            nc.vector.tensor_copy(ot[:, mc * 512:(mc + 1) * 512], op_[:])
        nc.sync.dma_start(outp[q0 + qs * 128:q0 + qs * 128 + 128, :], ot[:])

    AV_LAG = 3  # AV matmuls trail the scores/exp pipeline by this many chunks
    st_prev = None
    for c in range(4):  # q chunks of 512
        st = [sb.tile([128, 512], F32R, name=f"st{p}", tag=f"st{p}")
              for p in range(2)]
        # Interleave pair p's B-steps with the OTHER pair's A-blocks so the
        # (in-order) PE stream always has independent matmuls to chew on
        # while ACT drains the exps. AV matmuls trail their exp by AV_LAG
        # chunks so the PE never sits right behind ACT. Only one pair's AV
        # accumulators are live at a time (avsm tag has 2 slots).
        for p in range(2):
            av = [ps_sm.tile([DK + 1, 512], F32, name=f"av{hi}", tag="avsm")
                  for hi in range(2)]
            pend = []
            for j in range(4):
                for kc in range(4 * j, 4 * j + 4):
                    pend.append((kc, b_scores(p, c, kc)))
                    if len(pend) > AV_LAG:
                        kcp, eb = pend.pop(0)
                        b_av(p, kcp, eb, av)
                a_block(1 - p, c, j)
                if p == 0 and st_prev is not None:
                    o_block(c - 1, j, st_prev)
            for kcp, eb in pend:
                b_av(p, kcp, eb, av)
            b_finish(p, c, av, st[p])
        st_prev = st
    for qs in range(4):
        o_block(3, qs, st_prev)


_NC = None


def _get_program():
    global _NC
    if _NC is None:
        _NC = build_program()
    return _NC


def _shard_inputs(Q, K, V, Wq, bq, Wk, bk, Wv, bv, Wo, bo):
    c = np.ascontiguousarray
    f = np.float32
    in_maps = []
    # cache per-batch transposes (shared by 4 cores each)
    qt_b = [c(np.asarray(Q[b], f).T) for b in range(B)]
    kt_b = [c(np.asarray(K[b], f).T) for b in range(B)]
    vt_b = [c(np.asarray(V[b], f).T) for b in range(B)]
    for core in range(N_CORES):
        b, hp = divmod(core, N_CORES // B)
        r = slice(hp * PF, (hp + 1) * PF)
        in_maps.append({
            "qt": qt_b[b], "kt": kt_b[b], "vt": vt_b[b],
            "wq": c(np.asarray(Wq, f)[r, :].T),
            "wk": c(np.asarray(Wk, f)[r, :].T),
            "wv": c(np.asarray(Wv, f)[r, :].T),
            "wo": c(np.asarray(Wo, f)[:, r].T),
            "bq": np.asarray(bq, f)[r].reshape(PF, 1).copy(),
            "bk": np.asarray(bk, f)[r].reshape(PF, 1).copy(),
            "bv": np.broadcast_to(np.asarray(bv, f)[r], (128, PF)).copy(),
        })
    return in_maps


def _run(trace=False, **inputs):
    nc = _get_program()
    in_maps = _shard_inputs(**inputs)
    res = run_bass_kernel_spmd(nc, in_maps, core_ids=list(range(N_CORES)), trace=trace)
    bo = np.asarray(inputs["bo"], np.float32)
    attn_full = np.empty((B, H, S, S), np.float32)
    out = np.zeros((B, S, D), np.float32)
    for core in range(N_CORES):
        b, hp = divmod(core, N_CORES // B)
        attn_full[b, hp * HPC:(hp + 1) * HPC] = res.results[core]["attn"]
        out[b] += res.results[core]["outp"]
    out += bo
    return (out, attn_full), res


def kernel(**inputs):
    (out, attn_full), _ = _run(**inputs)
    return out, attn_full


# revision 40
# speedup vs baseline: 1.0280x; 1.0280x over previous
"""Multi-head attention (B=2, S=2048, D=1024, H=16) on 8 TRN2 NeuronCores.

Sharding: core = (batch b, 4 consecutive heads). Each core computes its 4
heads' Q/K/V projections, full [S,S] softmax attention (written to the attn
output), attn@V, and a partial O-projection. Host sums the 4 per-batch
O-partials and adds bo.

Device layout notes:
- All matmul contractions sit on the SBUF partition axis, so the host passes
  transposed views: qt/kt/vt = X[b].T [D,S]; wq/wk/wv = W[rows_h,:].T [D,256];
  wo = Wo[:,cols_h].T [256,D].
- Scores are computed twice, in [q,k] layout (for the attn output; exp row
  sums come free via activation accum_out) and in [k,q] layout (for attn@V;
  row sums come free via a ones-column appended to v).
"""

import sys

sys.path.insert(0, "/opt/trn_rl_repo")

from contextlib import ExitStack

import numpy as np

import concourse.bacc as bacc
import concourse.bass as bass
import concourse.mybir as mybir
import concourse.tile as tile
from concourse.bass_utils import run_bass_kernel_spmd

F32 = mybir.dt.float32
F32R = mybir.dt.float32r
BF16 = mybir.dt.bfloat16
AF = mybir.ActivationFunctionType




B, S, D, H, DK = 2, 2048, 1024, 16, 64
N_CORES = 8
HPC = H * B // N_CORES  # 4 heads per core
PF = HPC * DK  # 256 projected features per core
SCALE = 1.0 / np.sqrt(DK)  # 0.125


def build_program():
    nc = bacc.Bacc("TRN2", target_bir_lowering=False, debug=False, num_devices=N_CORES)

    qt = nc.dram_tensor("qt", [D, S], F32, kind="ExternalInput").ap()
    kt = nc.dram_tensor("kt", [D, S], F32, kind="ExternalInput").ap()
    vt = nc.dram_tensor("vt", [D, S], F32, kind="ExternalInput").ap()
    wq = nc.dram_tensor("wq", [D, PF], F32, kind="ExternalInput").ap()
    wk = nc.dram_tensor("wk", [D, PF], F32, kind="ExternalInput").ap()
    wv = nc.dram_tensor("wv", [D, PF], F32, kind="ExternalInput").ap()
    wo = nc.dram_tensor("wo", [PF, D], F32, kind="ExternalInput").ap()
    bq = nc.dram_tensor("bq", [PF, 1], F32, kind="ExternalInput").ap()
    bk = nc.dram_tensor("bk", [PF, 1], F32, kind="ExternalInput").ap()
    bv = nc.dram_tensor("bv", [128, PF], F32, kind="ExternalInput").ap()
    attn = nc.dram_tensor("attn", [HPC, S, S], F32, kind="ExternalOutput").ap()
    outp = nc.dram_tensor("outp", [S, D], F32, kind="ExternalOutput").ap()

    with tile.TileContext(nc) as tc, ExitStack() as ctx:
        _attention(ctx, tc, qt, kt, vt, wq, wk, wv, wo, bq, bk, bv, attn, outp)

    nc.compile()
    return nc


def _attention(ctx, tc, qt, kt, vt, wq, wk, wv, wo, bq, bk, bv, attn, outp):
    nc = tc.nc
    NDC = D // 128  # 8 contraction chunks for projections

    const = ctx.enter_context(tc.tile_pool(name="const", bufs=1))
    # PSUM budget (8 banks): "scores" 3 x [128,1024] = 6 banks,
    # "avsm" 2 x [128,512] = 2 banks (v-proj, AV accumulators, O-proj).
    ps = ctx.enter_context(tc.tile_pool(name="ps", bufs=3, space="PSUM"))
    ps_sm = ctx.enter_context(tc.tile_pool(name="ps_sm", bufs=2, space="PSUM"))

    # ---------------- persistent tiles ----------------
    wo_r = const.tile([128, 2, D], F32R, name="wo_r")
    bq_sb = const.tile([128, 2, 1], F32, name="bq_sb")
    bk_sb = const.tile([128, 2, 1], F32, name="bk_sb")
    bv_sb = const.tile([128, PF], F32, name="bv_sb")
    nc.sync.dma_start(bq_sb[:], bq.rearrange("(c p) o -> p c o", p=128))
    nc.sync.dma_start(bk_sb[:], bk.rearrange("(c p) o -> p c o", p=128))
    nc.sync.dma_start(bv_sb[:], bv[:])

    # qT/kT: per head-pair p, [128 (2 heads x 64 feat), S] f32r.
    qT = [const.tile([128, S], F32R, name=f"qT{p}") for p in range(2)]
    kT = [const.tile([128, S], F32R, name=f"kT{p}") for p in range(2)]
    # v (+ ones col): per s-chunk of 128, [128 (seq), 4 heads, DK+1] bf16.
    vaug = [const.tile([128, HPC, DK + 1], BF16, name=f"vaug{sc}") for sc in range(16)]

    # ---------------- projections ----------------
    with tc.tile_pool(name="wraw", bufs=1) as wraw, \
         tc.tile_pool(name="wrnd", bufs=1) as wrnd, \
         tc.tile_pool(name="inp", bufs=4) as inp, \
         tc.tile_pool(name="inpr", bufs=12) as inpr:

        def load_round(name, dram_ap, shape, raw_pool=None, rnd_pool=None,
                       tag=None):
            """DMA f32 into a raw tile, round to f32r on the (otherwise idle
            during projections) ScalarEngine."""
            raw_pool = raw_pool or inp
            rnd_pool = rnd_pool or inpr
            raw = raw_pool.tile(shape, F32, name=f"{name}w", tag=tag or f"{name}w")
            nc.sync.dma_start(raw[:], dram_ap)
            rnd = rnd_pool.tile(shape, F32R, name=name, tag=tag or name)
            nc.scalar.copy(rnd[:], raw[:])
            return rnd

        wq_r = load_round("wq_r", wq.rearrange("(c p) d -> p c d", p=128),
                          [128, NDC, PF], wraw, wrnd)
        wk_r = load_round("wk_r", wk.rearrange("(c p) d -> p c d", p=128),
                          [128, NDC, PF], wraw, wrnd)
        wv_r = load_round("wv_r", wv.rearrange("(c p) d -> p c d", p=128),
                          [128, NDC, PF], wraw, wrnd)
        wo_raw = wraw.tile([128, 2, D], F32, name="wo_raw")
        nc.sync.dma_start(wo_raw[:], wo.rearrange("(c p) m -> p c m", p=128))
        nc.scalar.copy(wo_r[:], wo_raw[:])

        # q/k projections, both head pairs at once per input chunk.
        for tsrc, wsb, bsb, dst in ((qt, wq_r, bq_sb, qT), (kt, wk_r, bk_sb, kT)):
            for sc in range(2):  # s chunks of 1024
                xt = [load_round(f"xr{dc}",
                                 tsrc[dc * 128:(dc + 1) * 128,
                                      sc * 1024:(sc + 1) * 1024],
                                 [128, 1024], tag="xr")
                      for dc in range(NDC)]
                for p in range(2):
                    pp = ps.tile([128, 1024], F32, name="pp", tag="scores")
                    for half in range(2):
                        for dc in range(NDC):
                            nc.tensor.matmul(
                                pp[:, half * 512:(half + 1) * 512],
                                wsb[:, dc, p * 128:(p + 1) * 128],
                                xt[dc][:, half * 512:half * 512 + 512],
                                start=(dc == 0), stop=(dc == NDC - 1),
                            )
                    nc.vector.tensor_scalar_add(
                        dst[p][:, sc * 1024:(sc + 1) * 1024], pp[:], bsb[:, p, :])

        # v projection: [s,256] = vt.T @ wv, all 4 heads at once.
        for sc2 in range(2):
            xt = [load_round(f"vr{dc}",
                             vt[dc * 128:(dc + 1) * 128,
                                sc2 * 1024:(sc2 + 1) * 1024],
                             [128, 1024], tag="xr")
                  for dc in range(NDC)]
            for ss in range(8):  # s sub-chunks of 128
                sc = sc2 * 8 + ss
                vp = ps_sm.tile([128, PF], F32, name="vp", tag="avsm")
                for dc in range(NDC):
                    nc.tensor.matmul(
                        vp[:], xt[dc][:, ss * 128:(ss + 1) * 128], wv_r[:, dc, :],
                        start=(dc == 0), stop=(dc == NDC - 1),
                    )
                nc.vector.memset(vaug[sc][:, :, DK:DK + 1], 1.0)
                nc.vector.tensor_tensor(
                    vaug[sc][:, :, 0:DK],
                    vp.rearrange("p (h d) -> p h d", d=DK),
                    bv_sb.rearrange("p (h d) -> p h d", d=DK),
                    op=mybir.AluOpType.add,
                )

    sb = ctx.enter_context(tc.tile_pool(name="sb", bufs=2))
    ebp = ctx.enter_context(tc.tile_pool(name="ebp", bufs=6))
    eap = ctx.enter_context(tc.tile_pool(name="eap", bufs=3))
    vec = ctx.enter_context(tc.tile_pool(name="vec", bufs=4))

    # ---------------- main loop ----------------
    def b_scores(p, c, kc):
        """Packed scores.T for both heads of pair p at k-chunk kc + exp."""
        q0 = c * 512
        sT = ps.tile([128, 1024], F32, name="sT", tag="scores")
        for hi in range(2):
            nc.tensor.matmul(
                sT[:, hi * 512:(hi + 1) * 512],
                kT[p][hi * 64:(hi + 1) * 64, kc * 128:(kc + 1) * 128],
                qT[p][hi * 64:(hi + 1) * 64, q0:q0 + 512],
                start=True, stop=True,
                tile_position=(hi * 64, 0),
            )
        ebT = ebp.tile([128, 2, 512], BF16, name="ebT", tag="ebT")
        nc.scalar.activation(
            ebT[:], sT.rearrange("p (h q) -> p h q", h=2), AF.Exp, scale=SCALE)
        return ebT

    def b_av(p, kc, ebT, av):
        """AV accumulation matmuls for k-chunk kc (one chunk behind scores)."""
        for hi in range(2):
            nc.tensor.matmul(
                av[hi][:], vaug[kc][:, p * 2 + hi, :], ebT[:, hi, :],
                start=(kc == 0), stop=(kc == 15),
            )

    def b_finish(p, c, av, st):
        """Normalize attn@V into the stacked st tile for pair p."""
        for hi in range(2):
            rT = vec.tile([1, 512], F32, name="rT", tag="rT")
            nc.vector.reciprocal(rT[:], av[hi][DK:DK + 1, :])
            rTb = vec.tile([DK, 512], F32, name="rTb", tag="rTb", bufs=2)
            nc.gpsimd.partition_broadcast(rTb[:], rT[:])
            nc.vector.tensor_tensor(
                st[hi * 64:(hi + 1) * 64, :], av[hi][0:DK, :],
                rTb[:], op=mybir.AluOpType.mult)

    def a_block(p, c, qs):
        """A-phase for both heads of pair p at q sub-chunk qs (packed
        matmuls): scores [q,k] -> exp (+rowsum) -> normalize -> attn out."""
        r0 = c * 512 + qs * 128
        ea = [eap.tile([128, S], BF16, name=f"ea{hi}", tag=f"ea{hi}")
              for hi in range(2)]
        rs = [vec.tile([128, 2], F32, name=f"rs{hi}", tag=f"rs{hi}")
              for hi in range(2)]
        for k4 in range(2):  # k chunks of 1024
            sA = [ps.tile([128, 1024], F32, name=f"sA{hi}", tag="scores")
                  for hi in range(2)]
            for kh in range(2):
                k0 = k4 * 1024 + kh * 512
                for hi in range(2):
                    nc.tensor.matmul(
                        sA[hi][:, kh * 512:(kh + 1) * 512],
                        qT[p][hi * 64:(hi + 1) * 64, r0:r0 + 128],
                        kT[p][hi * 64:(hi + 1) * 64, k0:k0 + 512],
                        start=True, stop=True,
                        tile_position=(hi * 64, 0),
                    )
            for hi in range(2):
                nc.scalar.activation(
                    ea[hi][:, k4 * 1024:(k4 + 1) * 1024], sA[hi][:], AF.Exp,
                    scale=SCALE, accum_out=rs[hi][:, k4:k4 + 1])
        for hi in range(2):
            h = p * 2 + hi
            rsum = vec.tile([128, 1], F32, name="rsum", tag="rsum")
            nc.vector.tensor_reduce(
                rsum[:], rs[hi][:], axis=mybir.AxisListType.X,
                op=mybir.AluOpType.add)
            rinv = vec.tile([128, 1], F32, name="rinv", tag="rinv")
            nc.vector.reciprocal(rinv[:], rsum[:])
            at = eap.tile([128, S], F32, name=f"at{hi}", tag=f"at{hi}", bufs=2)
            nc.vector.tensor_scalar_mul(at[:], ea[hi][:], rinv[:])
            nc.sync.dma_start(attn[h, r0:r0 + 128, :], at[:])

    def o_block(c, qs, st):
        """O-projection for q sub-chunk qs: out_partial += sum_p st_p.T@wo_p."""
        q0 = c * 512
        ot = sb.tile([128, D], F32, name="ot", tag="ot")
        for mc in range(2):
            op_ = ps_sm.tile([128, 512], F32, name="op_", tag="avsm")
            for p in range(2):
                nc.tensor.matmul(
                    op_[:], st[p][:, qs * 128:(qs + 1) * 128],
                    wo_r[:, p, mc * 512:(mc + 1) * 512],
                    start=(p == 0), stop=(p == 1),
                )
            nc.vector.tensor_copy(ot[:, mc * 512:(mc + 1) * 512], op_[:])
        nc.sync.dma_start(outp[q0 + qs * 128:q0 + qs * 128 + 128, :], ot[:])

    AV_LAG = 3  # AV matmuls trail the scores/exp pipeline by this many chunks
    st_prev = None
    for c in range(4):  # q chunks of 512
        st = [sb.tile([128, 512], F32R, name=f"st{p}", tag=f"st{p}")
              for p in range(2)]
        # Interleave pair p's B-steps with the OTHER pair's A-blocks so the
        # (in-order) PE stream always has independent matmuls to chew on
        # while ACT drains the exps. AV matmuls trail their exp by AV_LAG
        # chunks so the PE never sits right behind ACT. Only one pair's AV
        # accumulators are live at a time (avsm tag has 2 slots).
        for p in range(2):
            av = [ps_sm.tile([DK + 1, 512], F32, name=f"av{hi}", tag="avsm")
                  for hi in range(2)]
            pend = []
            for j in range(4):
                for kc in range(4 * j, 4 * j + 4):
                    pend.append((kc, b_scores(p, c, kc)))
                    if len(pend) > AV_LAG:
                        kcp, eb = pend.pop(0)
                        b_av(p, kcp, eb, av)
                a_block(1 - p, c, j)
                if p == 0 and st_prev is not None:
                    o_block(c - 1, j, st_prev)
            for kcp, eb in pend:
                b_av(p, kcp, eb, av)
            b_finish(p, c, av, st[p])
        st_prev = st
    for qs in range(4):
        o_block(3, qs, st_prev)


_NC = None


def _get_program():
    global _NC
    if _NC is None:
        _NC = build_program()
    return _NC


def _shard_inputs(Q, K, V, Wq, bq, Wk, bk, Wv, bv, Wo, bo):
    c = np.ascontiguousarray
    f = np.float32
    in_maps = []
    # cache per-batch transposes (shared by 4 cores each)
    qt_b = [c(np.asarray(Q[b], f).T) for b in range(B)]
    kt_b = [c(np.asarray(K[b], f).T) for b in range(B)]
    vt_b = [c(np.asarray(V[b], f).T) for b in range(B)]
    for core in range(N_CORES):
        b, hp = divmod(core, N_CORES // B)
        r = slice(hp * PF, (hp + 1) * PF)
        in_maps.append({
            "qt": qt_b[b], "kt": kt_b[b], "vt": vt_b[b],
            "wq": c(np.asarray(Wq, f)[r, :].T),
            "wk": c(np.asarray(Wk, f)[r, :].T),
            "wv": c(np.asarray(Wv, f)[r, :].T),
            "wo": c(np.asarray(Wo, f)[:, r].T),
            "bq": np.asarray(bq, f)[r].reshape(PF, 1).copy(),
            "bk": np.asarray(bk, f)[r].reshape(PF, 1).copy(),
            "bv": np.broadcast_to(np.asarray(bv, f)[r], (128, PF)).copy(),
        })
    return in_maps


def _run(trace=False, **inputs):
    nc = _get_program()
    in_maps = _shard_inputs(**inputs)
    res = run_bass_kernel_spmd(nc, in_maps, core_ids=list(range(N_CORES)), trace=trace)
    bo = np.asarray(inputs["bo"], np.float32)
    attn_full = np.empty((B, H, S, S), np.float32)
    out = np.zeros((B, S, D), np.float32)
    for core in range(N_CORES):
        b, hp = divmod(core, N_CORES // B)
        attn_full[b, hp * HPC:(hp + 1) * HPC] = res.results[core]["attn"]
        out[b] += res.results[core]["outp"]
    out += bo
    return (out, attn_full), res


def kernel(**inputs):
    (out, attn_full), _ = _run(**inputs)
    return out, attn_full


# revision 42
# speedup vs baseline: 1.1185x; 1.0881x over previous
"""Multi-head attention (B=2, S=2048, D=1024, H=16) on 8 TRN2 NeuronCores.

Sharding: core = (batch b, 4 consecutive heads). Each core computes its 4
heads' Q/K/V projections, full [S,S] softmax attention (written to the attn
output), attn@V, and a partial O-projection. Host sums the 4 per-batch
O-partials and adds bo.

Device layout notes:
- All matmul contractions sit on the SBUF partition axis, so the host passes
  transposed views: qt/kt/vt = X[b].T [D,S]; wq/wk/wv = W[rows_h,:].T [D,256];
  wo = Wo[:,cols_h].T [256,D].
- Scores are computed twice, in [q,k] layout (for the attn output; exp row
  sums come free via activation accum_out) and in [k,q] layout (for attn@V;
  row sums come free via a ones-column appended to v).
"""

import sys

sys.path.insert(0, "/opt/trn_rl_repo")

from contextlib import ExitStack

import numpy as np

import concourse.bacc as bacc
import concourse.bass as bass
import concourse.mybir as mybir
import concourse.tile as tile
from concourse.bass_utils import run_bass_kernel_spmd

F32 = mybir.dt.float32
F32R = mybir.dt.float32r
BF16 = mybir.dt.bfloat16
AF = mybir.ActivationFunctionType




B, S, D, H, DK = 2, 2048, 1024, 16, 64
N_CORES = 8
HPC = H * B // N_CORES  # 4 heads per core
PF = HPC * DK  # 256 projected features per core
SCALE = 1.0 / np.sqrt(DK)  # 0.125


def build_program():
    nc = bacc.Bacc("TRN2", target_bir_lowering=False, debug=False, num_devices=N_CORES)

    qt = nc.dram_tensor("qt", [D, S], F32, kind="ExternalInput").ap()
    kt = nc.dram_tensor("kt", [D, S], F32, kind="ExternalInput").ap()
    vt = nc.dram_tensor("vt", [D, S], F32, kind="ExternalInput").ap()
    wq = nc.dram_tensor("wq", [D, PF], F32, kind="ExternalInput").ap()
    wk = nc.dram_tensor("wk", [D, PF], F32, kind="ExternalInput").ap()
    wv = nc.dram_tensor("wv", [D, PF], F32, kind="ExternalInput").ap()
    wo = nc.dram_tensor("wo", [PF, D], F32, kind="ExternalInput").ap()
    bq = nc.dram_tensor("bq", [PF, 1], F32, kind="ExternalInput").ap()
    bk = nc.dram_tensor("bk", [PF, 1], F32, kind="ExternalInput").ap()
    bv = nc.dram_tensor("bv", [128, PF], F32, kind="ExternalInput").ap()
    attn = nc.dram_tensor("attn", [HPC, S, S], F32, kind="ExternalOutput").ap()
    outp = nc.dram_tensor("outp", [S, D], F32, kind="ExternalOutput").ap()

    with tile.TileContext(nc) as tc, ExitStack() as ctx:
        _attention(ctx, tc, qt, kt, vt, wq, wk, wv, wo, bq, bk, bv, attn, outp)

    nc.compile()
    return nc


def _attention(ctx, tc, qt, kt, vt, wq, wk, wv, wo, bq, bk, bv, attn, outp):
    nc = tc.nc
    NDC = D // 128  # 8 contraction chunks for projections

    const = ctx.enter_context(tc.tile_pool(name="const", bufs=1))
    # PSUM budget (8 banks): "scores" 3 x [128,1024] = 6 banks,
    # "avsm" 2 x [128,512] = 2 banks (v-proj, AV accumulators, O-proj).
    ps = ctx.enter_context(tc.tile_pool(name="ps", bufs=3, space="PSUM"))
    ps_sm = ctx.enter_context(tc.tile_pool(name="ps_sm", bufs=2, space="PSUM"))

    # ---------------- persistent tiles ----------------
    wo_r = const.tile([128, 2, D], F32R, name="wo_r")
    bq_sb = const.tile([128, 2, 1], F32, name="bq_sb")
    bk_sb = const.tile([128, 2, 1], F32, name="bk_sb")
    bv_sb = const.tile([128, PF], F32, name="bv_sb")
    nc.sync.dma_start(bq_sb[:], bq.rearrange("(c p) o -> p c o", p=128))
    nc.sync.dma_start(bk_sb[:], bk.rearrange("(c p) o -> p c o", p=128))
    nc.sync.dma_start(bv_sb[:], bv[:])

    # qT/kT: per head-pair p, [128 (2 heads x 64 feat), S] f32r.
    qT = [const.tile([128, S], F32R, name=f"qT{p}") for p in range(2)]
    kT = [const.tile([128, S], F32R, name=f"kT{p}") for p in range(2)]
    # v (+ ones col): per s-chunk of 128, [128 (seq), 4 heads, DK+1] bf16.
    vaug = [const.tile([128, HPC, DK + 1], BF16, name=f"vaug{sc}") for sc in range(16)]

    # ---------------- projections ----------------
    with tc.tile_pool(name="wraw", bufs=1) as wraw, \
         tc.tile_pool(name="wrnd", bufs=1) as wrnd, \
         tc.tile_pool(name="inp", bufs=4) as inp, \
         tc.tile_pool(name="inpr", bufs=12) as inpr:

        def load_round(name, dram_ap, shape, raw_pool=None, rnd_pool=None,
                       tag=None):
            """DMA f32 into a raw tile, round to f32r on the (otherwise idle
            during projections) ScalarEngine."""
            raw_pool = raw_pool or inp
            rnd_pool = rnd_pool or inpr
            raw = raw_pool.tile(shape, F32, name=f"{name}w", tag=tag or f"{name}w")
            nc.sync.dma_start(raw[:], dram_ap)
            rnd = rnd_pool.tile(shape, F32R, name=name, tag=tag or name)
            nc.scalar.copy(rnd[:], raw[:])
            return rnd

        wq_r = load_round("wq_r", wq.rearrange("(c p) d -> p c d", p=128),
                          [128, NDC, PF], wraw, wrnd)
        wk_r = load_round("wk_r", wk.rearrange("(c p) d -> p c d", p=128),
                          [128, NDC, PF], wraw, wrnd)
        wv_r = load_round("wv_r", wv.rearrange("(c p) d -> p c d", p=128),
                          [128, NDC, PF], wraw, wrnd)
        wo_raw = wraw.tile([128, 2, D], F32, name="wo_raw")
        nc.sync.dma_start(wo_raw[:], wo.rearrange("(c p) m -> p c m", p=128))
        nc.scalar.copy(wo_r[:], wo_raw[:])

        # q/k projections, both head pairs at once per input chunk.
        for tsrc, wsb, bsb, dst in ((qt, wq_r, bq_sb, qT), (kt, wk_r, bk_sb, kT)):
            for sc in range(2):  # s chunks of 1024
                xt = [load_round(f"xr{dc}",
                                 tsrc[dc * 128:(dc + 1) * 128,
                                      sc * 1024:(sc + 1) * 1024],
                                 [128, 1024], tag="xr")
                      for dc in range(NDC)]
                for p in range(2):
                    pp = ps.tile([128, 1024], F32, name="pp", tag="scores")
                    for half in range(2):
                        for dc in range(NDC):
                            nc.tensor.matmul(
                                pp[:, half * 512:(half + 1) * 512],
                                wsb[:, dc, p * 128:(p + 1) * 128],
                                xt[dc][:, half * 512:half * 512 + 512],
                                start=(dc == 0), stop=(dc == NDC - 1),
                            )
                    nc.vector.tensor_scalar_add(
                        dst[p][:, sc * 1024:(sc + 1) * 1024], pp[:], bsb[:, p, :])

        # v projection: [s,256] = vt.T @ wv, all 4 heads at once.
        for sc2 in range(2):
            xt = [load_round(f"vr{dc}",
                             vt[dc * 128:(dc + 1) * 128,
                                sc2 * 1024:(sc2 + 1) * 1024],
                             [128, 1024], tag="xr")
                  for dc in range(NDC)]
            for ss in range(8):  # s sub-chunks of 128
                sc = sc2 * 8 + ss
                vp = ps_sm.tile([128, PF], F32, name="vp", tag="avsm")
                for dc in range(NDC):
                    nc.tensor.matmul(
                        vp[:], xt[dc][:, ss * 128:(ss + 1) * 128], wv_r[:, dc, :],
                        start=(dc == 0), stop=(dc == NDC - 1),
                    )
                nc.vector.memset(vaug[sc][:, :, DK:DK + 1], 1.0)
                nc.vector.tensor_tensor(
                    vaug[sc][:, :, 0:DK],
                    vp.rearrange("p (h d) -> p h d", d=DK),
                    bv_sb.rearrange("p (h d) -> p h d", d=DK),
                    op=mybir.AluOpType.add,
                )

    sb = ctx.enter_context(tc.tile_pool(name="sb", bufs=2))
    ebp = ctx.enter_context(tc.tile_pool(name="ebp", bufs=6))
    eap = ctx.enter_context(tc.tile_pool(name="eap", bufs=3))
    vec = ctx.enter_context(tc.tile_pool(name="vec", bufs=4))

    # ---------------- main loop ----------------
    def b_scores(p, c, kc):
        """Packed scores.T for both heads of pair p at k-chunk kc + exp."""
        q0 = c * 512
        sT = ps.tile([128, 1024], F32, name="sT", tag="scores")
        for hi in range(2):
            nc.tensor.matmul(
                sT[:, hi * 512:(hi + 1) * 512],
                kT[p][hi * 64:(hi + 1) * 64, kc * 128:(kc + 1) * 128],
                qT[p][hi * 64:(hi + 1) * 64, q0:q0 + 512],
                start=True, stop=True,
                tile_position=(hi * 64, 0),
            )
        ebT = ebp.tile([128, 2, 512], BF16, name="ebT", tag="ebT")
        nc.scalar.activation(
            ebT[:], sT.rearrange("p (h q) -> p h q", h=2), AF.Exp, scale=SCALE)
        return ebT

    def b_av(p, kc, ebT, av):
        """AV accumulation matmuls for k-chunk kc (one chunk behind scores)."""
        for hi in range(2):
            nc.tensor.matmul(
                av[hi][:], vaug[kc][:, p * 2 + hi, :], ebT[:, hi, :],
                start=(kc == 0), stop=(kc == 15),
            )

    def b_finish(p, c, av, st):
        """Normalize attn@V into the stacked st tile for pair p."""
        for hi in range(2):
            rT = vec.tile([1, 512], F32, name="rT", tag="rT")
            nc.vector.reciprocal(rT[:], av[hi][DK:DK + 1, :])
            rTb = vec.tile([DK, 512], F32, name="rTb", tag="rTb", bufs=2)
            nc.gpsimd.partition_broadcast(rTb[:], rT[:])
            nc.vector.tensor_tensor(
                st[hi * 64:(hi + 1) * 64, :], av[hi][0:DK, :],
                rTb[:], op=mybir.AluOpType.mult)

    def a_block(p, c, qs):
        """A-phase for both heads of pair p at q sub-chunk qs (packed
        matmuls): scores [q,k] -> exp (+rowsum) -> normalize -> attn out."""
        r0 = c * 512 + qs * 128
        ea = [eap.tile([128, S], F32, name=f"ea{hi}", tag=f"ea{hi}")
              for hi in range(2)]
        rs = [vec.tile([128, 2], F32, name=f"rs{hi}", tag=f"rs{hi}")
              for hi in range(2)]
        for k4 in range(2):  # k chunks of 1024
            sA = [ps.tile([128, 1024], F32, name=f"sA{hi}", tag="scores")
                  for hi in range(2)]
            for kh in range(2):
                k0 = k4 * 1024 + kh * 512
                for hi in range(2):
                    nc.tensor.matmul(
                        sA[hi][:, kh * 512:(kh + 1) * 512],
                        qT[p][hi * 64:(hi + 1) * 64, r0:r0 + 128],
                        kT[p][hi * 64:(hi + 1) * 64, k0:k0 + 512],
                        start=True, stop=True,
                        tile_position=(hi * 64, 0),
                    )
            for hi in range(2):
                nc.scalar.activation(
                    ea[hi][:, k4 * 1024:(k4 + 1) * 1024], sA[hi][:], AF.Exp,
                    scale=SCALE, accum_out=rs[hi][:, k4:k4 + 1])
        for hi in range(2):
            h = p * 2 + hi
            rsum = vec.tile([128, 1], F32, name="rsum", tag="rsum")
            nc.vector.tensor_reduce(
                rsum[:], rs[hi][:], axis=mybir.AxisListType.X,
                op=mybir.AluOpType.add)
            rinv = vec.tile([128, 1], F32, name="rinv", tag="rinv")
            nc.vector.reciprocal(rinv[:], rsum[:])
            at = eap.tile([128, S], F32, name=f"at{hi}", tag=f"at{hi}", bufs=2)
            nc.vector.tensor_scalar_mul(at[:], ea[hi][:], rinv[:])
            nc.sync.dma_start(attn[h, r0:r0 + 128, :], at[:])

    def o_block(c, qs, st):
        """O-projection for q sub-chunk qs: out_partial += sum_p st_p.T@wo_p."""
        q0 = c * 512
        ot = sb.tile([128, D], F32, name="ot", tag="ot")
        for mc in range(2):
            op_ = ps_sm.tile([128, 512], F32, name="op_", tag="avsm")
            for p in range(2):
                nc.tensor.matmul(
                    op_[:], st[p][:, qs * 128:(qs + 1) * 128],
                    wo_r[:, p, mc * 512:(mc + 1) * 512],
                    start=(p == 0), stop=(p == 1),
                )
            nc.vector.tensor_copy(ot[:, mc * 512:(mc + 1) * 512], op_[:])
        nc.sync.dma_start(outp[q0 + qs * 128:q0 + qs * 128 + 128, :], ot[:])

    AV_LAG = 2  # AV matmuls trail the scores/exp pipeline by this many chunks
    st_prev = None
    for c in range(4):  # q chunks of 512
        st = [sb.tile([128, 512], F32R, name=f"st{p}", tag=f"st{p}")
              for p in range(2)]
        # Interleave pair p's B-steps with the OTHER pair's A-blocks so the
        # (in-order) PE stream always has independent matmuls to chew on
        # while ACT drains the exps. AV matmuls trail their exp by AV_LAG
        # chunks so the PE never sits right behind ACT. Only one pair's AV
        # accumulators are live at a time (avsm tag has 2 slots).
        for p in range(2):
            av = [ps_sm.tile([DK + 1, 512], F32, name=f"av{hi}", tag="avsm")
                  for hi in range(2)]
            pend = []
            for j in range(4):
                for kc in range(4 * j, 4 * j + 4):
                    pend.append((kc, b_scores(p, c, kc)))
                    if len(pend) > AV_LAG:
                        kcp, eb = pend.pop(0)
                        b_av(p, kcp, eb, av)
                a_block(1 - p, c, j)
                if p == 0 and st_prev is not None:
                    o_block(c - 1, j, st_prev)
            for kcp, eb in pend:
                b_av(p, kcp, eb, av)
            b_finish(p, c, av, st[p])
        st_prev = st
    for qs in range(4):
        o_block(3, qs, st_prev)


_NC = None


def _get_program():
    global _NC
    if _NC is None:
        _NC = build_program()
    return _NC


def _shard_inputs(Q, K, V, Wq, bq, Wk, bk, Wv, bv, Wo, bo):
    c = np.ascontiguousarray
    f = np.float32
    in_maps = []
    # cache per-batch transposes (shared by 4 cores each)
    qt_b = [c(np.asarray(Q[b], f).T) for b in range(B)]
    kt_b = [c(np.asarray(K[b], f).T) for b in range(B)]
    vt_b = [c(np.asarray(V[b], f).T) for b in range(B)]
    for core in range(N_CORES):
        b, hp = divmod(core, N_CORES // B)
        r = slice(hp * PF, (hp + 1) * PF)
        in_maps.append({
            "qt": qt_b[b], "kt": kt_b[b], "vt": vt_b[b],
            "wq": c(np.asarray(Wq, f)[r, :].T),
            "wk": c(np.asarray(Wk, f)[r, :].T),
            "wv": c(np.asarray(Wv, f)[r, :].T),
            "wo": c(np.asarray(Wo, f)[:, r].T),
            "bq": np.asarray(bq, f)[r].reshape(PF, 1).copy(),
            "bk": np.asarray(bk, f)[r].reshape(PF, 1).copy(),
            "bv": np.broadcast_to(np.asarray(bv, f)[r], (128, PF)).copy(),
        })
    return in_maps


def _run(trace=False, **inputs):
    nc = _get_program()
    in_maps = _shard_inputs(**inputs)
    res = run_bass_kernel_spmd(nc, in_maps, core_ids=list(range(N_CORES)), trace=trace)
    bo = np.asarray(inputs["bo"], np.float32)
    attn_full = np.empty((B, H, S, S), np.float32)
    out = np.zeros((B, S, D), np.float32)
    for core in range(N_CORES):
        b, hp = divmod(core, N_CORES // B)
        attn_full[b, hp * HPC:(hp + 1) * HPC] = res.results[core]["attn"]
        out[b] += res.results[core]["outp"]
    out += bo
    return (out, attn_full), res


def kernel(**inputs):
    (out, attn_full), _ = _run(**inputs)
    return out, attn_full
